# revision 11
# baseline (speedup 1.0000x reference)
"""MMoE-style CustomizedGateControl kernel for 8x TRN2 NeuronCores.

Data-parallel over the batch dim (16384 -> 8 x 2048). Per core, a
per-tile pipeline (16 b-tiles of 128 rows):
  - expert GEMMs with batch rows on PSUM partitions (12 experts + gates
    fused in one wide fp16 sweep)
  - bias add split between gpsimd and DVE, ReLU on the scalar engine
  - gated combine on DVE in 4x perf mode: per-partition tensor_scalar_mul
    with the gate column + an fp16 add tree (replaces the PE diag trick)
  - 4 small PE identity-transposes per tile flip combined info to
    expert-major for the tower GEMMs; towers run per 4-tile group
All parameters replicated; no collectives.
"""

import sys

if "/opt/trn_rl_repo" not in sys.path:
    sys.path.insert(0, "/opt/trn_rl_repo")

import numpy as np

import concourse.bacc as bacc
import concourse.mybir as mybir
import concourse.tile as tile
from concourse.bass_utils import run_bass_kernel_spmd

# problem dims
B, D, E, H = 16384, 512, 256, 128
S, K, T = 4, 4, 2
NCORES = 8
BC = B // NCORES          # 2048 batch rows per core
P = 128                   # partitions
NB = BC // P              # 16 b-tiles per core
NE = S + T * K            # 12 experts
G = S + K                 # 8 gate inputs per task
WCOLS = NE * E            # 3072 expert output columns
WALL = WCOLS + T * G      # 3088 = experts + gate columns
KC = D // P               # 4 contraction chunks
NTH = WCOLS // 512        # 6 sweep thirds per tile

f32 = mybir.dt.float32
f16 = mybir.dt.float16


def _src_col(t: int, g: int) -> int:
    """Column offset in a batch-major expert tile for gate input g of task t."""
    if g < S:
        return g * E                  # shared expert g
    return (S + t * K + (g - S)) * E  # task expert (t, g-S)


def _build():
    nc = bacc.Bacc("TRN2", target_bir_lowering=False, debug=False)

    xt_d = nc.dram_tensor("xt", [D, BC], f16, kind="ExternalInput").ap()
    wall_d = nc.dram_tensor("wall", [D, WALL], f16, kind="ExternalInput").ap()
    biasb_d = nc.dram_tensor("biasb", [P, WCOLS], f16, kind="ExternalInput").ap()
    tw1_d = nc.dram_tensor("tw1", [T, E, H], f16, kind="ExternalInput").ap()
    tb1_d = nc.dram_tensor("tb1", [H, T], f32, kind="ExternalInput").ap()
    tw2_d = nc.dram_tensor("tw2", [H, T], f16, kind="ExternalInput").ap()
    ident_d = nc.dram_tensor("ident", [P, P], f16, kind="ExternalInput").ap()
    out_d = nc.dram_tensor("out", [T, BC], f32, kind="ExternalOutput").ap()

    with tile.TileContext(nc) as tc:
        with (
            tc.tile_pool(name="const", bufs=1) as const,
            tc.tile_pool(name="expsb", bufs=3) as expsb_pool,
            tc.tile_pool(name="tmp", bufs=2) as tmp_pool,
            tc.tile_pool(name="info", bufs=2) as info_pool,
            tc.tile_pool(name="hsb", bufs=2) as hsb_pool,
        ):
            # ---- persistent inputs ----
            xt_t = [const.tile([P, BC], f16, tag=f"xt{k}", name=f"xt{k}") for k in range(KC)]
            wall_t = [const.tile([P, WALL], f16, tag=f"wall{k}", name=f"wall{k}") for k in range(KC)]
            biasb = const.tile([P, WCOLS], f16, tag="biasb", name="biasb")
            ident = const.tile([P, P], f16, tag="ident", name="ident")
            tb1 = const.tile([H, T], f32, tag="tb1", name="tb1")
            tw2 = const.tile([H, T], f16, tag="tw2", name="tw2")
            tw1_t = {}
            for t in range(T):
                for kc in range(2):
                    t_ = const.tile([P, H], f16, tag=f"tw1_{t}_{kc}", name=f"tw1_{t}_{kc}")
                    tw1_t[(t, kc)] = t_
            gsb = [
                const.tile([P, T * G], f32, tag=f"gsb{i}", name=f"gsb{i}")
                for i in range(NB)
            ]
            # info transposed, expert-dim on partitions; col = q*128 + b_low
            # with q = tile*4 + t*2 + ec
            infoT = const.tile([P, 4 * BC], f16, tag="infoT", name="infoT")
            out_sb = const.tile([1, T * BC], f32, tag="out_sb", name="out_sb")

            # ---- input streaming (consumption order; sync/scalar HWDGE) ----
            for k in range(KC):
                nc.sync.dma_start(xt_t[k][:, 0:256], xt_d[k * P : (k + 1) * P, 0:256])
            for k in range(KC):
                nc.sync.dma_start(
                    wall_t[k][:, 0:1024], wall_d[k * P : (k + 1) * P, 0:1024]
                )
                nc.scalar.dma_start(
                    wall_t[k][:, WCOLS:WALL], wall_d[k * P : (k + 1) * P, WCOLS:WALL]
                )
            nc.scalar.dma_start(biasb[:, 0:1024], biasb_d[:, 0:1024])
            for k in range(KC):
                nc.sync.dma_start(
                    wall_t[k][:, 1024:2048], wall_d[k * P : (k + 1) * P, 1024:2048]
                )
            nc.scalar.dma_start(biasb[:, 1024:2048], biasb_d[:, 1024:2048])
            for k in range(KC):
                nc.sync.dma_start(
                    wall_t[k][:, 2048:WCOLS], wall_d[k * P : (k + 1) * P, 2048:WCOLS]
                )
            nc.scalar.dma_start(biasb[:, 2048:WCOLS], biasb_d[:, 2048:WCOLS])
            nc.scalar.dma_start(ident[:], ident_d[:])
            nc.scalar.dma_start(tb1[:], tb1_d[:])
            nc.scalar.dma_start(tw2[:], tw2_d[:])
            for t in range(T):
                for kc in range(2):
                    nc.scalar.dma_start(
                        tw1_t[(t, kc)][:], tw1_d[t, kc * P : (kc + 1) * P, :]
                    )
            for k in range(KC):
                nc.sync.dma_start(xt_t[k][:, 256:BC], xt_d[k * P : (k + 1) * P, 256:BC])

            with (
                tc.tile_pool(name="eps", bufs=2, space="PSUM") as eps_pool,
                tc.tile_pool(name="gps", bufs=1, space="PSUM") as gps_pool,
                tc.tile_pool(name="ct", bufs=1, space="PSUM") as ct_pool,
                tc.tile_pool(name="hps", bufs=1, space="PSUM") as hps_pool,
                tc.tile_pool(name="ops", bufs=1, space="PSUM") as ops_pool,
            ):
                gps = gps_pool.tile([P, NB * T * G], f32, tag="gps", name="gps")
                exp_sb = {}
                info_t = {}

                def emit_sweep(i):
                    bs = slice(i * P, (i + 1) * P)
                    esb = expsb_pool.tile([P, WCOLS], f16, tag="expsb", name=f"expsb{i}")
                    exp_sb[i] = esb
                    for pair in range(3):
                        c0 = pair * 1024
                        eps = eps_pool.tile([P, 1024], f32, tag="eps", name="eps")
                        for half in range(2):
                            h0 = half * 512
                            for k in range(KC):
                                nc.tensor.matmul(
                                    eps[:, h0 : h0 + 512],
                                    xt_t[k][:, bs],
                                    wall_t[k][:, c0 + h0 : c0 + h0 + 512],
                                    start=(k == 0),
                                    stop=(k == KC - 1),
                                )
                            # interleave gate matmuls (k = 0,1 after pair 0;
                            # 2 after pair 1; 3 after pair 2)
                            gmap = {(0, 0): 0, (0, 1): 1, (1, 1): 2, (2, 1): 3}
                            if (pair, half) in gmap:
                                k = gmap[(pair, half)]
                                gs = slice(i * T * G, (i + 1) * T * G)
                                nc.tensor.matmul(
                                    gps[:, gs],
                                    xt_t[k][:, bs],
                                    wall_t[k][:, WCOLS:WALL],
                                    start=(k == 0),
                                    stop=(k == KC - 1),
                                )
                        # bias add on DVE; exp_sb holds PRE-relu values (the
                        # relu is fused into the combine's tensor_scalar max)
                        nc.vector.tensor_add(
                            esb[:, c0 : c0 + 1024], eps[:], biasb[:, c0 : c0 + 1024]
                        )
                    nc.scalar.copy(
                        gsb[i][:], gps[:, i * T * G : (i + 1) * T * G]
                    )

                def emit_combine(i):
                    esb = exp_sb.pop(i)
                    tmp = tmp_pool.tile([P, G * T * E], f16, tag="tmp", name="tmp")
                    info = info_pool.tile([P, T * E], f16, tag="info", name=f"info{i}")
                    info_t[i] = info
                    for g in range(G):
                        for t in range(T):
                            sc = _src_col(t, g)
                            # fused relu + gate: (esb max 0) * gate[:, (t,g)]
                            nc.vector.tensor_scalar(
                                tmp[:, g * 512 + t * E : g * 512 + (t + 1) * E],
                                esb[:, sc : sc + E],
                                0.0,
                                gsb[i][:, g * T + t : g * T + t + 1],
                                op0=mybir.AluOpType.max,
                                op1=mybir.AluOpType.mult,
                            )
                    # fp16 add tree over g (both tasks per op); first two
                    # levels on gpsimd (SBUF-only), final add on DVE
                    for a in range(4):
                        nc.gpsimd.tensor_add(
                            tmp[:, a * 1024 : a * 1024 + 512],
                            tmp[:, a * 1024 : a * 1024 + 512],
                            tmp[:, a * 1024 + 512 : a * 1024 + 1024],
                        )
                    nc.gpsimd.tensor_add(tmp[:, 0:512], tmp[:, 0:512], tmp[:, 1024:1536])
                    nc.gpsimd.tensor_add(
                        tmp[:, 2048:2560], tmp[:, 2048:2560], tmp[:, 3072:3584]
                    )
                    nc.vector.tensor_add(info[:], tmp[:, 0:512], tmp[:, 2048:2560])
                    # PE identity-transposes: info [128b, (q)*128] -> infoT
                    ct = ct_pool.tile([P, 512], f32, tag="ct", name="ct")
                    for q in range(4):
                        nc.tensor.matmul(
                            ct[:, q * P : (q + 1) * P],
                            info[:, q * P : (q + 1) * P],
                            ident[:],
                            start=True,
                            stop=True,
                        )
                        nc.scalar.copy(
                            infoT[:, (i * 4 + q) * P : (i * 4 + q + 1) * P],
                            ct[:, q * P : (q + 1) * P],
                        )

                def emit_towers(gr):
                    gt0 = gr * 4
                    b0 = gt0 * P
                    iT = infoT[:].rearrange("p (gt q c) -> p gt q c", q=4, c=P)
                    for t in range(T):
                        hp = hps_pool.tile([P, 512], f32, tag="hps", name="hps")
                        for kc in range(2):
                            nc.tensor.matmul(
                                hp[:],
                                tw1_t[(t, kc)][:],
                                iT[:, gt0 : gt0 + 4, t * 2 + kc, :],
                                start=(kc == 0),
                                stop=(kc == 1),
                            )
                        hs = hsb_pool.tile([P, 512], f16, tag="hsb", name="hsb")
                        nc.scalar.activation(
                            hs[:],
                            hp[:],
                            mybir.ActivationFunctionType.Relu,
                            bias=tb1[:, t : t + 1],
                        )
                        ops = ops_pool.tile([1, 512], f32, tag="ops", name="ops")
                        nc.tensor.matmul(
                            ops[0:1, :],
                            tw2[:, t : t + 1],
                            hs[:],
                            start=True,
                            stop=True,
                        )
                        nc.scalar.copy(
                            out_sb[0:1, t * BC + b0 : t * BC + b0 + 512], ops[0:1, :]
                        )
                        nc.gpsimd.dma_start(
                            out_d.rearrange("t n -> (t n)")[
                                None, t * BC + b0 : t * BC + b0 + 512
                            ],
                            out_sb[0:1, t * BC + b0 : t * BC + b0 + 512],
                        )

                for i in range(NB):
                    emit_sweep(i)
                    if i >= 1:
                        emit_combine(i - 1)
                        j = i - 1
                        if j >= 4 and (j - 4) % 4 == 0:
                            emit_towers((j - 4) // 4)
                emit_combine(NB - 1)
                emit_towers(3)

    nc.compile()
    return nc


_NC = None


def _get_nc():
    global _NC
    if _NC is None:
        _NC = _build()
    return _NC


def _prep_shared(shared_W, shared_b, task_W, task_b, gate_W, tower_W1, tower_b1, tower_W2):
    cols = [np.asarray(shared_W[s]) for s in range(S)]
    cols += [np.asarray(task_W[t, k]) for t in range(T) for k in range(K)]
    gwi = np.empty((D, T * G), np.float32)
    for t in range(T):
        gwi[:, t::T] = np.asarray(gate_W[t])  # column g*T+t = gate (t, g)
    cols += [gwi]
    wall = np.ascontiguousarray(np.concatenate(cols, axis=1), dtype=np.float16)
    bias_all = np.concatenate(
        [np.asarray(shared_b).reshape(-1), np.asarray(task_b).reshape(-1)]
    ).astype(np.float32)
    biasb = np.ascontiguousarray(np.broadcast_to(bias_all, (P, WCOLS)).astype(np.float16))
    tw1 = np.ascontiguousarray(tower_W1, dtype=np.float16)
    tb1 = np.ascontiguousarray(np.asarray(tower_b1).T, dtype=np.float32)   # [H, T]
    tw2 = np.ascontiguousarray(np.asarray(tower_W2)[:, :, 0].T, dtype=np.float16)  # [H, T]
    ident = np.eye(P, dtype=np.float16)
    return wall, biasb, tw1, tb1, tw2, ident


def kernel(
    x,
    shared_W,
    shared_b,
    task_W,
    task_b,
    gate_W,
    tower_W1,
    tower_b1,
    tower_W2,
    tower_b2,
    _trace=False,
    _tmpdir=None,
):
    nc = _get_nc()
    x = np.asarray(x, dtype=np.float32)
    wall, biasb, tw1, tb1, tw2, ident = _prep_shared(
        shared_W, shared_b, task_W, task_b, gate_W, tower_W1, tower_b1, tower_W2
    )
    in_maps = []
    for c in range(NCORES):
        xt = np.ascontiguousarray(x[c * BC : (c + 1) * BC, :].T.astype(np.float16))
        in_maps.append(
            {
                "xt": xt,
                "wall": wall,
                "biasb": biasb,
                "tw1": tw1,
                "tb1": tb1,
                "tw2": tw2,
                "ident": ident,
            }
        )
    kw = {}
    if _trace:
        kw = {"trace": True, "tmpdir": _tmpdir}
    res = run_bass_kernel_spmd(nc, in_maps, core_ids=list(range(NCORES)), **kw)
    out = np.concatenate([res.results[c]["out"] for c in range(NCORES)], axis=1)
    out = out + np.asarray(tower_b2, dtype=np.float32)[:, 0][:, None]
    result = out[:, :, None].astype(np.float32)  # [T, B, 1]
    if _trace:
        return result, res
    return result


# revision 14
# speedup vs baseline: 1.0574x; 1.0574x over previous
"""MMoE-style CustomizedGateControl kernel for 8x TRN2 NeuronCores.

Data-parallel over the batch dim (16384 -> 8 x 2048). Per core, a
per-tile pipeline (16 b-tiles of 128 rows):
  - expert GEMMs with batch rows on PSUM partitions (12 experts + gates
    fused in one wide fp16 sweep)
  - bias add split between gpsimd and DVE, ReLU on the scalar engine
  - gated combine on DVE in 4x perf mode: per-partition tensor_scalar_mul
    with the gate column + an fp16 add tree (replaces the PE diag trick)
  - 4 small PE identity-transposes per tile flip combined info to
    expert-major for the tower GEMMs; towers run per 4-tile group
All parameters replicated; no collectives.
"""

import sys

if "/opt/trn_rl_repo" not in sys.path:
    sys.path.insert(0, "/opt/trn_rl_repo")

import numpy as np

import concourse.bacc as bacc
import concourse.mybir as mybir
import concourse.tile as tile
from concourse.bass_utils import run_bass_kernel_spmd

# problem dims
B, D, E, H = 16384, 512, 256, 128
S, K, T = 4, 4, 2
NCORES = 8
BC = B // NCORES          # 2048 batch rows per core
P = 128                   # partitions
NB = BC // P              # 16 b-tiles per core
NE = S + T * K            # 12 experts
G = S + K                 # 8 gate inputs per task
WCOLS = NE * E            # 3072 expert output columns
WALL = WCOLS + T * G      # 3088 = experts + gate columns
KC = D // P               # 4 contraction chunks
NTH = WCOLS // 512        # 6 sweep thirds per tile

f32 = mybir.dt.float32
f16 = mybir.dt.float16


def _src_col(t: int, g: int) -> int:
    """Column offset in a batch-major expert tile for gate input g of task t."""
    if g < S:
        return g * E                  # shared expert g
    return (S + t * K + (g - S)) * E  # task expert (t, g-S)


def _build():
    nc = bacc.Bacc("TRN2", target_bir_lowering=False, debug=False)

    xt_d = nc.dram_tensor("xt", [D, BC], f16, kind="ExternalInput").ap()
    wall_d = nc.dram_tensor("wall", [D, WALL], f16, kind="ExternalInput").ap()
    biasb_d = nc.dram_tensor("biasb", [P, WCOLS], f16, kind="ExternalInput").ap()
    tw1_d = nc.dram_tensor("tw1", [T, E, H], f16, kind="ExternalInput").ap()
    tb1_d = nc.dram_tensor("tb1", [H, T], f32, kind="ExternalInput").ap()
    tw2_d = nc.dram_tensor("tw2", [H, T], f16, kind="ExternalInput").ap()
    ident_d = nc.dram_tensor("ident", [P, P], f16, kind="ExternalInput").ap()
    out_d = nc.dram_tensor("out", [T, BC], f32, kind="ExternalOutput").ap()

    with tile.TileContext(nc) as tc:
        with (
            tc.tile_pool(name="const", bufs=1) as const,
            tc.tile_pool(name="expsb", bufs=3) as expsb_pool,
            tc.tile_pool(name="tmp", bufs=3) as tmp_pool,
            tc.tile_pool(name="info", bufs=2) as info_pool,
            tc.tile_pool(name="hsb", bufs=2) as hsb_pool,
        ):
            # ---- persistent inputs ----
            xt_t = [const.tile([P, BC], f16, tag=f"xt{k}", name=f"xt{k}") for k in range(KC)]
            wall_t = [const.tile([P, WALL], f16, tag=f"wall{k}", name=f"wall{k}") for k in range(KC)]
            biasb = const.tile([P, WCOLS], f16, tag="biasb", name="biasb")
            ident = const.tile([P, P], f16, tag="ident", name="ident")
            tb1 = const.tile([H, T], f32, tag="tb1", name="tb1")
            tw2 = const.tile([H, T], f16, tag="tw2", name="tw2")
            tw1_t = {}
            for t in range(T):
                for kc in range(2):
                    t_ = const.tile([P, H], f16, tag=f"tw1_{t}_{kc}", name=f"tw1_{t}_{kc}")
                    tw1_t[(t, kc)] = t_
            gsb = [
                const.tile([P, T * G], f32, tag=f"gsb{i}", name=f"gsb{i}")
                for i in range(NB)
            ]
            # info transposed, expert-dim on partitions; col = q*128 + b_low
            # with q = tile*4 + t*2 + ec
            infoT = const.tile([P, 4 * BC], f16, tag="infoT", name="infoT")
            out_sb = const.tile([1, T * BC], f32, tag="out_sb", name="out_sb")

            # ---- input streaming (consumption order; sync/scalar HWDGE) ----
            for k in range(KC):
                nc.sync.dma_start(xt_t[k][:, 0:256], xt_d[k * P : (k + 1) * P, 0:256])
            for k in range(KC):
                nc.sync.dma_start(
                    wall_t[k][:, 0:1024], wall_d[k * P : (k + 1) * P, 0:1024]
                )
                nc.scalar.dma_start(
                    wall_t[k][:, WCOLS:WALL], wall_d[k * P : (k + 1) * P, WCOLS:WALL]
                )
            nc.scalar.dma_start(biasb[:, 0:1024], biasb_d[:, 0:1024])
            for k in range(KC):
                nc.sync.dma_start(
                    wall_t[k][:, 1024:2048], wall_d[k * P : (k + 1) * P, 1024:2048]
                )
            nc.scalar.dma_start(biasb[:, 1024:2048], biasb_d[:, 1024:2048])
            for k in range(KC):
                nc.sync.dma_start(
                    wall_t[k][:, 2048:WCOLS], wall_d[k * P : (k + 1) * P, 2048:WCOLS]
                )
            nc.scalar.dma_start(biasb[:, 2048:WCOLS], biasb_d[:, 2048:WCOLS])
            nc.scalar.dma_start(ident[:], ident_d[:])
            nc.scalar.dma_start(tb1[:], tb1_d[:])
            nc.scalar.dma_start(tw2[:], tw2_d[:])
            for t in range(T):
                for kc in range(2):
                    nc.scalar.dma_start(
                        tw1_t[(t, kc)][:], tw1_d[t, kc * P : (kc + 1) * P, :]
                    )
            for k in range(KC):
                nc.sync.dma_start(xt_t[k][:, 256:BC], xt_d[k * P : (k + 1) * P, 256:BC])

            with (
                tc.tile_pool(name="eps", bufs=2, space="PSUM") as eps_pool,
                tc.tile_pool(name="gps", bufs=1, space="PSUM") as gps_pool,
                tc.tile_pool(name="ct", bufs=1, space="PSUM") as ct_pool,
                tc.tile_pool(name="hps", bufs=1, space="PSUM") as hps_pool,
                tc.tile_pool(name="ops", bufs=1, space="PSUM") as ops_pool,
            ):
                gps = gps_pool.tile([P, NB * T * G], f32, tag="gps", name="gps")
                exp_sb = {}
                info_t = {}

                def emit_sweep(i):
                    bs = slice(i * P, (i + 1) * P)
                    esb = expsb_pool.tile([P, WCOLS], f16, tag="expsb", name=f"expsb{i}")
                    exp_sb[i] = esb
                    for pair in range(3):
                        c0 = pair * 1024
                        eps = eps_pool.tile([P, 1024], f32, tag="eps", name="eps")
                        for half in range(2):
                            h0 = half * 512
                            for k in range(KC):
                                nc.tensor.matmul(
                                    eps[:, h0 : h0 + 512],
                                    xt_t[k][:, bs],
                                    wall_t[k][:, c0 + h0 : c0 + h0 + 512],
                                    start=(k == 0),
                                    stop=(k == KC - 1),
                                )
                            # interleave gate matmuls (k = 0,1 after pair 0;
                            # 2 after pair 1; 3 after pair 2)
                            gmap = {(0, 0): 0, (0, 1): 1, (1, 1): 2, (2, 1): 3}
                            if (pair, half) in gmap:
                                k = gmap[(pair, half)]
                                gs = slice(i * T * G, (i + 1) * T * G)
                                nc.tensor.matmul(
                                    gps[:, gs],
                                    xt_t[k][:, bs],
                                    wall_t[k][:, WCOLS:WALL],
                                    start=(k == 0),
                                    stop=(k == KC - 1),
                                )
                        # drain psum->fp16 on ACT, then in-place fp16 bias add
                        # on DVE (2x mode); exp_sb holds PRE-relu values (the
                        # relu is fused into the combine's tensor_scalar max)
                        nc.scalar.copy(esb[:, c0 : c0 + 1024], eps[:])
                        nc.vector.tensor_add(
                            esb[:, c0 : c0 + 1024],
                            esb[:, c0 : c0 + 1024],
                            biasb[:, c0 : c0 + 1024],
                        )
                    nc.scalar.copy(
                        gsb[i][:], gps[:, i * T * G : (i + 1) * T * G]
                    )

                def emit_combine(i):
                    esb = exp_sb.pop(i)
                    tmp = tmp_pool.tile([P, G * T * E], f16, tag="tmp", name="tmp")
                    info = info_pool.tile([P, T * E], f16, tag="info", name=f"info{i}")
                    info_t[i] = info
                    for g in range(G):
                        for t in range(T):
                            sc = _src_col(t, g)
                            # fused relu + gate: (esb max 0) * gate[:, (t,g)]
                            # g-even terms land in tmp[0:2048], g-odd in
                            # tmp[2048:4096] so the add tree uses 3 wide ops
                            dc = (g // 2) * 512 + t * E + (g % 2) * 2048
                            nc.vector.tensor_scalar(
                                tmp[:, dc : dc + E],
                                esb[:, sc : sc + E],
                                0.0,
                                gsb[i][:, g * T + t : g * T + t + 1],
                                op0=mybir.AluOpType.max,
                                op1=mybir.AluOpType.mult,
                            )
                    # fp16 add tree: one wide gpsimd level, two DVE levels
                    nc.gpsimd.tensor_add(
                        tmp[:, 0:2048], tmp[:, 0:2048], tmp[:, 2048:4096]
                    )
                    nc.vector.tensor_add(
                        tmp[:, 0:1024], tmp[:, 0:1024], tmp[:, 1024:2048]
                    )
                    nc.vector.tensor_add(info[:], tmp[:, 0:512], tmp[:, 512:1024])
                    # PE identity-transposes: info [128b, (q)*128] -> infoT
                    ct = ct_pool.tile([P, 512], f32, tag="ct", name="ct")
                    for q in range(4):
                        nc.tensor.matmul(
                            ct[:, q * P : (q + 1) * P],
                            info[:, q * P : (q + 1) * P],
                            ident[:],
                            start=True,
                            stop=True,
                        )
                        nc.scalar.copy(
                            infoT[:, (i * 4 + q) * P : (i * 4 + q + 1) * P],
                            ct[:, q * P : (q + 1) * P],
                        )

                def emit_towers(gr):
                    gt0 = gr * 4
                    b0 = gt0 * P
                    iT = infoT[:].rearrange("p (gt q c) -> p gt q c", q=4, c=P)
                    for t in range(T):
                        hp = hps_pool.tile([P, 512], f32, tag="hps", name="hps")
                        for kc in range(2):
                            nc.tensor.matmul(
                                hp[:],
                                tw1_t[(t, kc)][:],
                                iT[:, gt0 : gt0 + 4, t * 2 + kc, :],
                                start=(kc == 0),
                                stop=(kc == 1),
                            )
                        hs = hsb_pool.tile([P, 512], f16, tag="hsb", name="hsb")
                        nc.scalar.activation(
                            hs[:],
                            hp[:],
                            mybir.ActivationFunctionType.Relu,
                            bias=tb1[:, t : t + 1],
                        )
                        ops = ops_pool.tile([1, 512], f32, tag="ops", name="ops")
                        nc.tensor.matmul(
                            ops[0:1, :],
                            tw2[:, t : t + 1],
                            hs[:],
                            start=True,
                            stop=True,
                        )
                        nc.scalar.copy(
                            out_sb[0:1, t * BC + b0 : t * BC + b0 + 512], ops[0:1, :]
                        )
                        nc.gpsimd.dma_start(
                            out_d.rearrange("t n -> (t n)")[
                                None, t * BC + b0 : t * BC + b0 + 512
                            ],
                            out_sb[0:1, t * BC + b0 : t * BC + b0 + 512],
                        )

                for i in range(NB):
                    emit_sweep(i)
                    if i >= 1:
                        emit_combine(i - 1)
                        j = i - 1
                        if j >= 4 and (j - 4) % 4 == 0:
                            emit_towers((j - 4) // 4)
                emit_combine(NB - 1)
                emit_towers(3)

    nc.compile()
    return nc


_NC = None


def _get_nc():
    global _NC
    if _NC is None:
        _NC = _build()
    return _NC


def _prep_shared(shared_W, shared_b, task_W, task_b, gate_W, tower_W1, tower_b1, tower_W2):
    cols = [np.asarray(shared_W[s]) for s in range(S)]
    cols += [np.asarray(task_W[t, k]) for t in range(T) for k in range(K)]
    gwi = np.empty((D, T * G), np.float32)
    for t in range(T):
        gwi[:, t::T] = np.asarray(gate_W[t])  # column g*T+t = gate (t, g)
    cols += [gwi]
    wall = np.ascontiguousarray(np.concatenate(cols, axis=1), dtype=np.float16)
    bias_all = np.concatenate(
        [np.asarray(shared_b).reshape(-1), np.asarray(task_b).reshape(-1)]
    ).astype(np.float32)
    biasb = np.ascontiguousarray(np.broadcast_to(bias_all, (P, WCOLS)).astype(np.float16))
    tw1 = np.ascontiguousarray(tower_W1, dtype=np.float16)
    tb1 = np.ascontiguousarray(np.asarray(tower_b1).T, dtype=np.float32)   # [H, T]
    tw2 = np.ascontiguousarray(np.asarray(tower_W2)[:, :, 0].T, dtype=np.float16)  # [H, T]
    ident = np.eye(P, dtype=np.float16)
    return wall, biasb, tw1, tb1, tw2, ident


def kernel(
    x,
    shared_W,
    shared_b,
    task_W,
    task_b,
    gate_W,
    tower_W1,
    tower_b1,
    tower_W2,
    tower_b2,
    _trace=False,
    _tmpdir=None,
):
    nc = _get_nc()
    x = np.asarray(x, dtype=np.float32)
    wall, biasb, tw1, tb1, tw2, ident = _prep_shared(
        shared_W, shared_b, task_W, task_b, gate_W, tower_W1, tower_b1, tower_W2
    )
    in_maps = []
    for c in range(NCORES):
        xt = np.ascontiguousarray(x[c * BC : (c + 1) * BC, :].T.astype(np.float16))
        in_maps.append(
            {
                "xt": xt,
                "wall": wall,
                "biasb": biasb,
                "tw1": tw1,
                "tb1": tb1,
                "tw2": tw2,
                "ident": ident,
            }
        )
    kw = {}
    if _trace:
        kw = {"trace": True, "tmpdir": _tmpdir}
    res = run_bass_kernel_spmd(nc, in_maps, core_ids=list(range(NCORES)), **kw)
    out = np.concatenate([res.results[c]["out"] for c in range(NCORES)], axis=1)
    out = out + np.asarray(tower_b2, dtype=np.float32)[:, 0][:, None]
    result = out[:, :, None].astype(np.float32)  # [T, B, 1]
    if _trace:
        return result, res
    return result
